# revision 12
# baseline (speedup 1.0000x reference)
"""Trainium2 Bass kernel for nn_CapsShapeLayer (capsule dynamic routing).

Reference computation:
    u_hat[b,r,c,o] = sum_i W[r,c,o,i] * x[b,r,i]        (151 MB if materialized)
    3 routing iterations:
        c = softmax(b_logits, axis=r)
        s[b,c,o] = sum_r c[r,c] * u_hat[b,r,c,o]
        v = squash(s)                                    (elementwise)
        b_logits += mean_b <u_hat[b,r,c,:], v[b,c,:]>

Kernel strategy (u_hat is never materialized):
  * Shard R=1152 across the 8 cores (144 routes each; K_local = 144*8 = 1152
    = 9 partition tiles of 128 in the fused (r,i) contraction dim).
  * s~[b,(c,o)] = sum_{(r,i)} exp(b)[r,c] * Wt[(r,i),(c,o)] * Xt[(r,i),b]
    -- a K=1152 matmul per core with the softmax divide deferred past the
    cross-core reduction:  s = s~ / S,  S[c] = global sum_r exp(b[r,c]).
  * One AllReduce per iteration carries both the s~ partial [128,256] and
    the 16 exp-sums. Only iterations 0 and 1 need a device-side collective:
    the last iteration's partials go straight to the output and the final
    sum + squash happen in the host-side unshard.
  * All inputs and matmul operands are bf16; collective payloads are fp16
    (CCE adds in fp16). Measured end-to-end error ~3e-3 vs 2e-2 tolerance.
  * The post-AllReduce body is pipelined per 2-tile chunk (G -> P -> RED ->
    a -> b -> exp -> c_part -> Wc -> s~ accumulate) so PE/DVE/ACT overlap;
    PSUM->bf16 casts run on the Scalar engine so DVE multiplies stay 16-bit.
  * squash via  v = s|s| / (1+s^2) = s~|s~| / (S^2 + s~^2); the reciprocal
    is a single-instruction DVE approximation and the sign comes from
    ScalarE.
  * agreement: a[r,c] = sum_{o,i} W * G with G = Xb^T @ (v/B) -- a K=128
    matmul per (r,i)-tile, then an elementwise W*G and an 8->1 partition
    reduction done as a 0/1-matrix matmul (col-tiled into the right rows).
  * b_logits stays r-sharded per core; iteration 0 (b=0 -> uniform c) skips
    the exp/scale work entirely.
"""

import sys

for _p in ("/opt/trn_rl_repo",):
    if _p not in sys.path:
        sys.path.insert(0, _p)

import numpy as np
import ml_dtypes

import concourse.bass as bass
import concourse.bacc as bacc
import concourse.mybir as mybir
import concourse.tile as tile
from concourse.bass_utils import run_bass_kernel_spmd

F32 = mybir.dt.float32
F16 = mybir.dt.float16
BF16 = mybir.dt.bfloat16
AX = mybir.AxisListType
ALU = mybir.AluOpType
ACT = mybir.ActivationFunctionType

NPBF = ml_dtypes.bfloat16

B = 128          # batch
R = 1152         # routes (input capsules)
C = 16           # output capsules
O = 16           # output capsule dim
I = 8            # input capsule dim
CO = C * O       # 256
NCORES = 8
RS = R // NCORES          # 144 local routes
KL = RS * I               # 1152 local contraction
KT = KL // 128            # 9 K-tiles of 128
NITER = 3
S0 = float(R)             # global softmax denom at iteration 0 (b == 0)

_CACHED = None


def _make_consts():
    """Constant matrices packed into one [128, 321] bf16 input.

    cols 0:32    RED_even[p, m] = 1 if m == p//8          (i-reduction, even tile)
    cols 32:64   RED_odd [p, m] = 1 if m == 16 + p//8     (i-reduction, odd tile)
    col  64      ones                                      (partition sum)
    cols 65:193  REP[r', p] = 1 if r' == p//8  (rows 0:16) (c -> (r,i) replicate)
    cols 193:321 RED16[p, m] = 1 if m == p//8, m < 16      (i-reduction, tile 8)
    """
    cst = np.zeros((128, 321), np.float32)
    p = np.arange(128)
    cst[p, p // 8] = 1.0                     # RED_even
    cst[p, 32 + 16 + p // 8] = 1.0           # RED_odd
    cst[:, 64] = 1.0                         # ones
    cst[p // 8, 65 + p] = 1.0                # REP (row r'=p//8, col p)
    cst[p, 193 + p // 8] = 1.0               # RED16 (cols 193:209 nonzero)
    # RP[r_loc, 128*t + p] = 1 iff r_loc == 16*t + p//8   (PE operands must
    # sit at base partition 0, so each K-tile gets its own row-select matrix)
    rp = np.zeros((128, 8 * 128), np.float32)
    for t in range(8):
        rp[16 * t + p // 8, 128 * t + p] = 1.0
    return cst.astype(NPBF), rp.astype(NPBF)


def _prep_inputs(x, W):
    """Host-side shard + relayout. Returns list of 8 per-core input dicts."""
    x = np.ascontiguousarray(np.asarray(x, dtype=np.float32))
    W = np.ascontiguousarray(np.asarray(W, dtype=np.float32))
    Wr = W.reshape(R, C, O, I)
    cst, rp = _make_consts()
    in_maps = []
    for k in range(NCORES):
        sh = slice(k * RS, (k + 1) * RS)
        # Wt[(r,i),(c,o)] tile-interleaved to [p, t, co]
        wt = Wr[sh].transpose(0, 3, 1, 2).reshape(KL, CO)
        wt = np.ascontiguousarray(
            wt.reshape(KT, 128, CO).transpose(1, 0, 2).astype(NPBF)
        )
        # Xt[(r,i), b] tile-interleaved to [p, t, b]
        xt = x[:, sh, :].transpose(1, 2, 0).reshape(KL, B)
        xt = np.ascontiguousarray(
            xt.reshape(KT, 128, B).transpose(1, 0, 2).astype(NPBF)
        )
        # Xb[b, (r,i)] natural
        xb = np.ascontiguousarray(x[:, sh, :].reshape(B, KL).astype(NPBF))
        in_maps.append(
            {"wt_in": wt, "xt_in": xt, "xb_in": xb, "cst_in": cst, "rp_in": rp}
        )
    return in_maps


def _build_nc():
    nc = bacc.Bacc(
        "TRN2",
        target_bir_lowering=False,
        debug=False,
        num_devices=NCORES,
    )
    wt_d = nc.dram_tensor("wt_in", [128, KT, CO], BF16, kind="ExternalInput")
    xt_d = nc.dram_tensor("xt_in", [128, KT, B], BF16, kind="ExternalInput")
    xb_d = nc.dram_tensor("xb_in", [B, KL], BF16, kind="ExternalInput")
    cst_d = nc.dram_tensor("cst_in", [128, 321], BF16, kind="ExternalInput")
    rp_d = nc.dram_tensor("rp_in", [128, 8 * 128], BF16, kind="ExternalInput")
    v_d = nc.dram_tensor("v_out", [B, CO + C], F16, kind="ExternalOutput")

    rg = [list(range(NCORES))]

    with tile.TileContext(nc) as tc:
        with (
            tc.tile_pool(name="persist", bufs=1) as pp,
            tc.tile_pool(name="work", bufs=2) as wp,
            tc.tile_pool(name="ps_s", bufs=1, space="PSUM") as pool_ps_s,
            tc.tile_pool(name="ps_g", bufs=2, space="PSUM") as pool_ps_g,
            tc.tile_pool(name="ps_small", bufs=3, space="PSUM") as pool_ps_small,
            tc.tile_pool(name="ps_a", bufs=1, space="PSUM") as pool_ps_a,
            tc.tile_pool(name="dram", bufs=1, space="DRAM") as dp,
        ):
            # ---- persistent SBUF state ----
            wt_sb = pp.tile([128, KT, CO], BF16, name="wt_sb")
            wc_sb = pp.tile([128, KT, CO], BF16, name="wc_sb")
            xt_sb = pp.tile([128, KT, B], BF16, name="xt_sb")
            xb_sb = pp.tile([B, KL], BF16, name="xb_sb")
            cst_sb = pp.tile([128, 321], BF16, name="cst_sb")
            rp_sb = pp.tile([128, 8 * 128], BF16, name="rp_sb")
            b_sb = pp.tile([128, 2 * C], F32, name="b_sb")

            # wt/xt first: they gate the iteration-0 matmuls
            nc.sync.dma_start(wt_sb[:], wt_d[:])
            nc.sync.dma_start(xt_sb[:], xt_d[:])
            nc.sync.dma_start(xb_sb[:], xb_d[:])
            nc.sync.dma_start(cst_sb[:], cst_d[:])
            nc.sync.dma_start(rp_sb[:], rp_d[:])
            nc.vector.memset(b_sb[:], 0.0)

            red_even = cst_sb[:, 0:32]
            red_odd = cst_sb[:, 32:64]
            red16_pad = cst_sb[:, 193:321]
            ones_col = cst_sb[:, 64:65]
            rep = cst_sb[0:16, 65:193]

            mm = nc.tensor.matmul

            H = CO // 2

            def build_wc_pair(tile_list, eb, c_ps):
                """c_part matmuls + Wc scale for the given K-tiles.

                c_ps is a fresh PSUM tile [128, len(tile_list)*C]."""
                n = len(tile_list)
                for j, t in enumerate(tile_list):
                    if t < 8:
                        mm(
                            c_ps[:, C * j : C * (j + 1)],
                            rp_sb[:, 128 * t : 128 * (t + 1)],
                            eb[:, 0:C],
                            start=True, stop=True,
                        )
                    else:
                        mm(
                            c_ps[:, C * j : C * (j + 1)],
                            rep, eb[0:16, C : 2 * C],
                            start=True, stop=True,
                        )
                lo = tile_list[0]
                c_bf = wp.tile([128, 2 * C], BF16, name="c_bf", tag="c_bf")
                nc.scalar.copy(c_bf[:, 0 : n * C], c_ps[:])
                cb = c_bf[:, 0 : n * C].rearrange("p (t c) -> p t c", t=n)[:, :, :, None]
                nc.vector.tensor_mul(
                    wc_sb[:, lo : lo + n].rearrange("p t (c o) -> p t c o", c=C),
                    wt_sb[:, lo : lo + n].rearrange("p t (c o) -> p t c o", c=C),
                    cb.broadcast_to([128, n, C, O]),
                )

            for it in range(NITER):
                first, last = it == 0, it == NITER - 1
                ps_s = pool_ps_s.tile([B, CO], F32, name="ps_s", tag="s")

                if first:
                    # uniform c: s~0 straight from Wt
                    for t in range(KT):
                        mm(
                            ps_s[:], xt_sb[:, t, :], wt_sb[:, t, :],
                            start=(t == 0), stop=(t == KT - 1),
                        )
                    s_ps = None
                else:
                    # ---- squash of the previous iteration's AllReduce ----
                    # v = s~|s~| / (S^2 + s~^2), via halves so PE starts on
                    # half 0 while DVE squashes half 1
                    cc_out = cc_out_prev
                    eb = wp.tile([128, 2 * C], BF16, name="eb", tag="eb")
                    # c_part matmuls read all 128 eb partitions (zero-weighted
                    # outside their own rows); rows written by later chunks
                    # must be finite, not uninitialized SBUF. Memset before
                    # the squash so DVE does it while waiting on the AR.
                    nc.vector.memset(eb[:], 0.0)
                    if it == 1:
                        sS2 = None
                    else:
                        sS = wp.tile([128, C], F16, name="sS", tag="sS")
                        nc.sync.dma_start(
                            sS[:], cc_out[0:1, CO : CO + C].to_broadcast([128, C])
                        )
                        sS2 = wp.tile([128, C], BF16, name="sS2", tag="sS2")
                        nc.vector.tensor_mul(sS2[:], sS[:], sS[:])
                    s_sb = wp.tile([B, CO], F16, name="s_sb", tag="ssb")
                    qt = wp.tile([B, CO], BF16, name="qt", tag="qt")
                    # reciprocal_approx needs fp32 bit layout
                    q2 = wp.tile([B, CO], F32, name="q2", tag="q2")
                    rec = wp.tile([B, CO], F32, name="rec", tag="rec")
                    sg = wp.tile([B, CO], BF16, name="sg", tag="sg")
                    m = wp.tile([B, CO], BF16, name="m", tag="m")
                    vg = wp.tile([B, CO], BF16, name="vg", tag="vg")
                    for h in range(2):
                        sl = slice(H * h, H * (h + 1))
                        nc.sync.dma_start(s_sb[:, sl], cc_out[:, sl])
                        nc.vector.tensor_mul(qt[:, sl], s_sb[:, sl], s_sb[:, sl])
                        if it == 1:
                            nc.vector.tensor_scalar_add(q2[:, sl], qt[:, sl], S0 * S0)
                        else:
                            ch = slice(C // 2 * h, C // 2 * (h + 1))
                            nc.vector.tensor_add(
                                q2[:, sl].rearrange("b (c o) -> b c o", c=C // 2),
                                qt[:, sl].rearrange("b (c o) -> b c o", c=C // 2),
                                sS2[:, ch, None].broadcast_to([B, C // 2, O]),
                            )
                        nc.vector.reciprocal_approx_fast(rec[:, sl], q2[:, sl])
                        nc.scalar.activation(sg[:, sl], s_sb[:, sl], ACT.Sign)
                        nc.vector.tensor_mul(m[:, sl], qt[:, sl], sg[:, sl])
                        nc.vector.scalar_tensor_tensor(
                            vg[:, sl],
                            rec[:, sl], 1.0 / B, m[:, sl],
                            op0=ALU.mult, op1=ALU.mult,
                        )

                    # ---- pipelined agreement + Wc build + s~ accumulate ----
                    # per 2-tile chunk: G = Xb^T @ vg; P = Wt*G; a = RED^T @ P;
                    # b += a; eb = exp(b); c_part; Wc = Wt*c; s~ += Xt^T @ Wc.
                    # Chunk T's DVE work overlaps chunk T+1's PE work.
                    eb = wp.tile([128, 2 * C], BF16, name="eb", tag="eb")
                    # c_part matmuls read all 128 eb partitions (zero-weighted
                    # outside their own rows); rows written by later chunks
                    # must be finite, not uninitialized SBUF
                    nc.vector.memset(eb[:], 0.0)
                    p_sb = wp.tile([128, KT, CO], BF16, name="p_sb", tag="p_sb")
                    a_sb = wp.tile([128, 2 * C], F32, name="a_sb", tag="a_sb")
                    ps_a = pool_ps_a.tile([128, 2, C, O], F32, name="ps_a", tag="a")
                    chunks = [(0, 1), (2, 3), (4, 5), (6, 7), (8,)]
                    for ci, tl in enumerate(chunks):
                        nt = len(tl)
                        ps_g = pool_ps_g.tile([128, 2, CO], F32, name="ps_g", tag="g")
                        for j, t in enumerate(tl):
                            for h in range(2):
                                sl = slice(H * h, H * (h + 1))
                                mm(
                                    ps_g[:, j, sl],
                                    xb_sb[:, 128 * t : 128 * (t + 1)],
                                    vg[:, sl],
                                    start=True, stop=True,
                                )
                        c0 = tl[0]
                        # PSUM->bf16 cast on the (mostly idle) Scalar engine so
                        # the DVE multiply runs bf16 x bf16 at 16-bit rate
                        g_bf = wp.tile([128, 2, CO], BF16, name="g_bf", tag="g_bf")
                        nc.scalar.copy(g_bf[:, 0:nt, :], ps_g[:, 0:nt, :])
                        nc.vector.tensor_mul(
                            p_sb[:, c0 : c0 + nt, :],
                            wt_sb[:, c0 : c0 + nt, :],
                            g_bf[:, 0:nt, :],
                        )
                        if nt == 2:
                            T = c0 // 2
                            rows = slice(32 * T, 32 * (T + 1))
                            for j, t in enumerate(tl):
                                mm(
                                    ps_a[rows, 0],
                                    red_even if t % 2 == 0 else red_odd,
                                    p_sb[:, t, :],
                                    start=(j == 0), stop=(j == 1),
                                    tile_position=(0, 32 * T),
                                )
                            nc.vector.tensor_reduce(
                                a_sb[rows, 0:C], ps_a[rows, 0], axis=AX.X, op=ALU.add
                            )
                            nc.vector.tensor_add(
                                b_sb[rows, 0:C], b_sb[rows, 0:C], a_sb[rows, 0:C]
                            )
                            nc.scalar.activation(
                                eb[rows, 0:C], b_sb[rows, 0:C], ACT.Exp
                            )
                        else:
                            mm(
                                ps_a[:, 1], red16_pad, p_sb[:, 8, :],
                                start=True, stop=True,
                            )
                            nc.vector.tensor_reduce(
                                a_sb[0:16, C : 2 * C], ps_a[0:16, 1],
                                axis=AX.X, op=ALU.add,
                            )
                            nc.vector.tensor_add(
                                b_sb[0:16, C : 2 * C],
                                b_sb[0:16, C : 2 * C],
                                a_sb[0:16, C : 2 * C],
                            )
                            nc.scalar.activation(
                                eb[0:16, C : 2 * C],
                                b_sb[0:16, C : 2 * C],
                                ACT.Exp,
                            )
                        c_ps = pool_ps_small.tile(
                            [128, nt * C], F32, name=f"c_ps{ci}", tag="sp"
                        )
                        build_wc_pair(list(tl), eb, c_ps)
                        for j, t in enumerate(tl):
                            mm(
                                ps_s[:], xt_sb[:, t, :], wc_sb[:, t, :],
                                start=(t == 0), stop=(t == KT - 1),
                            )
                    # local softmax denominator S_loc[c] = sum_r eb[r, c]
                    s_ps = pool_ps_small.tile([1, C], F32, name="s_ps", tag="sp")
                    mm(
                        s_ps[:], ones_col[0:128, :], eb[:, 0:C],
                        start=True, stop=False,
                    )
                    mm(
                        s_ps[:], ones_col[0:16, :], eb[0:16, C : 2 * C],
                        start=False, stop=True,
                    )

                # bounce to DRAM (+ S_loc in cols 256:272); on the last
                # iteration ship the raw partial to the host instead of the
                # AllReduce -- the final sum + squash is host-side unshard
                ew = CO if first else CO + C
                st_sb = wp.tile([128, ew], F16, name="st_sb", tag=f"st{ew}")
                nc.scalar.copy(st_sb[:, 0:CO], ps_s[:])
                if not first:
                    nc.vector.memset(st_sb[:, CO:ew], 0.0)
                    nc.scalar.copy(st_sb[0:1, CO:ew], s_ps[:])
                if last:
                    nc.sync.dma_start(v_d[:], st_sb[:])
                    continue
                cc_in = dp.tile([128, ew], F16, name=f"cc_in{it}")
                cc_out_prev = dp.tile(
                    [128, ew], F16, name=f"cc_out{it}", addr_space="Shared"
                )
                nc.sync.dma_start(cc_in[:], st_sb[:])
                nc.gpsimd.collective_compute(
                    "AllReduce", ALU.add, replica_groups=rg,
                    ins=[cc_in[:].opt()], outs=[cc_out_prev[:].opt()],
                )

    nc.compile()
    return nc


def _get_nc():
    global _CACHED
    if _CACHED is None:
        _CACHED = _build_nc()
    return _CACHED


def _postprocess(outs):
    """Host-side unshard of the final routing iteration: sum the per-core
    s~ partials and exp-sums, then apply the deferred softmax divide and
    squash (v = s~|s~| / (S^2 + s~^2))."""
    tot = np.stack([np.asarray(o).astype(np.float32) for o in outs]).sum(axis=0)
    st = tot[:, 0:CO]
    S = tot[0, CO : CO + C]
    S2 = np.repeat(S * S, O)[None, :]
    q = st * st
    v = (q * np.sign(st) / (S2 + q)).astype(np.float32)
    return v.reshape(B, C, O)


def kernel(x, W):
    nc = _get_nc()
    in_maps = _prep_inputs(x, W)
    res = run_bass_kernel_spmd(nc, in_maps, list(range(NCORES)))
    return _postprocess([res.results[k]["v_out"] for k in range(NCORES)])
